# revision 1
# baseline (speedup 1.0000x reference)
"""CRF forward (log-partition) kernel for Trainium2, 8 NeuronCores.

Problem: T=16384 steps, NT=1024 tags.
  alpha_0 = strans + emit[0]
  alpha_t[k] = emit[t,k] + logsumexp_j(alpha_{t-1}[j] + trans[j,k])
  out = logsumexp(alpha_{T-1} + etrans) - gold_path_score

Algorithm (validated in fp64/bf16 numpy prototype + CoreSim):
  Work in exp space: with Mc[k]=max_j trans[j,k], Ehat=exp(trans-Mc) in (0,1],
  ghat_t = exp(emit[t]+Mc-mu_t), mu_t = max_k(emit[t]+Mc) + RBAR, the scan is a
  LINEAR recurrence  b_t = ghat_t * (Ehat^T b_{t-1})  whose scalar offsets are
  tracked exactly on the host.  Products of positive matrices contract
  directions (~30x/step here), so the sequence is cut into 1024 chunks of 16
  steps; each chunk's chain starts from an arbitrary positive vector W-16
  steps early (burn-in) after which its direction matches the true state's.
  The per-chunk unknown scalar is recovered on the host by telescoping ratios
  of dumped seam vectors.  Chain 0 is exact (starts from b_0, injected via an
  additive patch).  128 chains per core x 8 cores run batched: each micro-step
  is 64 matmuls [128x128 bf16 Ehat-block weights] x [128-chain moving operand]
  accumulating in PSUM, then 8 DVE multiplies by ghat.  W micro-steps total,
  no inter-core communication.

Chain schedule: chain i>=1 covers steps [16i, 16i+16); micro-step tau
processes step 16i + tau - (W-15) (burn-in tau <= W-16).  Chain 0 covers
steps 1..15: ghat=0 padding until tau=W-15, then a K=1 outer-product matmul
injects b_0 into PSUM (chain 0's ghat column is 1.0 at that step).  Each
chain's start vector is ghat of its first processed step, so the tau=1 g
tile doubles as the initial b (no separate bstart load).  Dumps (bf16
b-tiles): u at tau=W-16 (state entering own chunk), v at tau=W (state
entering next chunk).  Host fixup (fp64): C_0=0,
C_i = C_{i-1} + log sum(v_{i-1}) - log sum(u_i);
logZ = log(v_last . exp(etrans)) + C_last + c_off.
"""

import numpy as np

T, NT = 16384, 1024
NCORES = 8
CH = 128            # chains per core
W = 17              # micro-steps; burn-in = W-16 = 1 (seam err ~5e-8 rel,
                    # 100x below the bf16 noise floor; contraction ~20x/step)
PATCH_TAU = W - 15  # patch applied after this micro-step
U_TAU = W - 16
V_TAU = W
RBAR = 1.33         # per-step growth fold-in (keeps b in bf16 range)
N_WARMUP = 48       # junk matmuls to trip the HAM clock-gate during init DMA

_CACHE = {}


def _build_nc():
    import concourse.bass as bass
    import concourse.mybir as mybir
    import concourse.tile as tile
    from concourse import bacc

    nc = bacc.Bacc("TRN2", target_bir_lowering=False, debug=False,
                   num_devices=NCORES)
    bf16 = mybir.dt.bfloat16
    f32 = mybir.dt.float32

    EH = nc.dram_tensor("ehat", [8, 128, 1024], bf16, kind="ExternalInput")
    GH = nc.dram_tensor("ghat", [W, 128, 1024], bf16, kind="ExternalInput")
    PA = nc.dram_tensor("patch", [1, 1024], bf16, kind="ExternalInput")
    DU = nc.dram_tensor("du", [128, 1024], bf16, kind="ExternalOutput")
    DV = nc.dram_tensor("dv", [128, 1024], bf16, kind="ExternalOutput")

    with tile.TileContext(nc) as tc:
        with (
            tc.tile_pool(name="const", bufs=1) as const,
            tc.tile_pool(name="bpool", bufs=2) as bpool,
            tc.tile_pool(name="gpool", bufs=4) as gpool,
            tc.tile_pool(name="psum", bufs=6, space="PSUM") as psum,
            tc.tile_pool(name="wpsum", bufs=1, space="PSUM") as wpsum,
        ):
            # PE warm-up: junk matmuls with no DMA dependency so the HAM
            # un-throttles while the init DMAs stream in.
            wsrc = const.tile([128, 128], bf16)
            nc.any.memset(wsrc[:], 0.25)
            wps = wpsum.tile([128, 128], f32)
            for _ in range(N_WARMUP):
                nc.tensor.matmul(wps[:], lhsT=wsrc[:], rhs=wsrc[:],
                                 start=True, stop=True)

            # Init loads: ehat groups + first g tiles + patch, in parallel
            # (HWDGE shares bandwidth; explicit serialization loses ~1.5us
            # DMA-start latency per group — measured).  No separate bstart:
            # the start vector of each chain is ghat of its first processed
            # step, which IS the tau=1 g tile — alias it as the initial b.
            ehat_sb = []
            g_tiles = {}
            for mth in range(8):
                t = const.tile([128, 1024], bf16, tag=f"eh{mth}")
                nc.sync.dma_start(t[:], EH[mth])
                ehat_sb.append(t)
                if mth == 0:
                    for gi in (1, 2):
                        gt = gpool.tile([128, 1024], bf16, tag="g")
                        nc.sync.dma_start(gt[:], GH[gi - 1])
                        g_tiles[gi] = gt
            cur_b = g_tiles[1]
            # Patch is injected as a K=1 outer-product matmul into PSUM at
            # PATCH_TAU (ps += patch_row^T @ onehot), with the host setting
            # chain 0's ghat column to 1.0 at that step — avoids a serial
            # 1024-wide DVE add on the critical path.
            patch_sb = const.tile([1, 1024], bf16)
            nc.sync.dma_start(patch_sb[:], PA[:])
            onehot = const.tile([1, 128], bf16)
            nc.any.memset(onehot[:], 0.0)
            nc.any.memset(onehot[0:1, 0:1], 1.0)

            for tau in range(1, W + 1):
                g_tile = g_tiles.pop(tau)
                if tau + 2 <= W:
                    nt = gpool.tile([128, 1024], bf16, tag="g")
                    nc.sync.dma_start(nt[:], GH[tau + 1])
                    g_tiles[tau + 2] = nt
                new_b = bpool.tile([128, 1024], bf16, tag="b")
                for mth in range(8):
                    ps = psum.tile([128, 128], f32, tag="ps")
                    for jc in range(8):
                        nc.tensor.matmul(
                            ps[:],
                            lhsT=ehat_sb[mth][:, jc * 128:(jc + 1) * 128],
                            rhs=cur_b[:, jc * 128:(jc + 1) * 128],
                            start=(jc == 0),
                            stop=(jc == 7 and tau != PATCH_TAU),
                        )
                    if tau == PATCH_TAU:
                        nc.tensor.matmul(
                            ps[:],
                            lhsT=patch_sb[:, mth * 128:(mth + 1) * 128],
                            rhs=onehot[:],
                            start=False, stop=True)
                    nc.vector.tensor_tensor(
                        out=new_b[:, mth * 128:(mth + 1) * 128], in0=ps[:],
                        in1=g_tile[:, mth * 128:(mth + 1) * 128],
                        op=mybir.AluOpType.mult)
                if tau == U_TAU:
                    nc.sync.dma_start(DU[:], new_b[:])
                if tau == V_TAU:
                    nc.sync.dma_start(DV[:], new_b[:])
                cur_b = new_b

    nc.compile()
    return nc


def _get_nc():
    if "nc" not in _CACHE:
        _CACHE["nc"] = _build_nc()
    return _CACHE["nc"]


def _chain_steps():
    """steps[i, tau-1] = global step processed by chain i at micro-step tau
    (-1 = pad)."""
    steps = np.full((NCORES * CH, W), -1, dtype=np.int64)
    taus = np.arange(1, W + 1)
    steps[0] = np.where(taus > PATCH_TAU, taus - PATCH_TAU, -1)
    idx = np.arange(1, NCORES * CH)
    steps[1:] = 16 * idx[:, None] + taus[None, :] - PATCH_TAU
    return steps


def _preprocess(emit, trans, strans):
    import ml_dtypes
    bf16 = ml_dtypes.bfloat16

    emit64 = emit.astype(np.float64)
    trans64 = trans.astype(np.float64)
    Mc = trans64.max(axis=0)
    Ehat = np.exp(trans64 - Mc[None, :]).astype(np.float32)
    # grouped [mth, p, jc*128+q] = Ehat[jc*128+p, mth*128+q]
    eh = np.ascontiguousarray(
        Ehat.reshape(8, 128, 8, 128).transpose(2, 1, 0, 3).reshape(8, 128, 1024)
    ).astype(bf16)

    A = emit64 + Mc[None, :]
    mu = A.max(axis=1) + RBAR                       # [T]
    ghat = np.exp(A - mu[:, None]).astype(np.float32)   # [T, NT]

    a0 = strans.astype(np.float64) + emit64[0]
    c0 = a0.max()
    b0 = np.exp(a0 - c0).astype(np.float32)
    c_off = c0 + mu[1:].sum()

    steps = _chain_steps()
    in_maps = []
    for c in range(NCORES):
        S = steps[c * CH:(c + 1) * CH]              # [CH, W]
        G = ghat[np.clip(S, 0, T - 1)]              # [CH, W, NT]
        G = np.where((S >= 1)[:, :, None], G, 0.0)
        if c == 0:
            # chain 0 pad: ghat=1 at PATCH_TAU so the PSUM-injected patch
            # passes through the multiply unchanged
            G[0, PATCH_TAU - 1, :] = 1.0
        # GH[tau, p, mth*128+ch] = ghat[t_ch, mth*128+p]
        Gt = (G.transpose(1, 2, 0)                  # [W, NT, CH]
                .reshape(W, 8, 128, CH)
                .transpose(0, 2, 1, 3)
                .reshape(W, 128, 8 * CH))
        gh = np.ascontiguousarray(Gt).astype(bf16)

        pa = np.zeros((1, 1024), np.float32)
        if c == 0:
            pa[0] = b0                              # k-ordered patch row
        in_maps.append({"ehat": np.asarray(eh),
                        "ghat": np.asarray(gh),
                        "patch": pa.astype(bf16)})
    return in_maps, c_off


def _postprocess(results, etrans, c_off):
    """Telescoping seam corrections in fp64."""
    n = NCORES * CH
    Us = np.zeros(n)
    Vs = np.zeros(n)
    v_last = None
    for c in range(NCORES):
        du = results[c]["du"].astype(np.float64).reshape(128, 8, CH)
        dv = results[c]["dv"].astype(np.float64).reshape(128, 8, CH)
        Us[c * CH:(c + 1) * CH] = du.sum(axis=(0, 1))
        Vs[c * CH:(c + 1) * CH] = dv.sum(axis=(0, 1))
        if c == NCORES - 1:
            # v[k = mth*128+p] of last chain = dv[p, mth, CH-1]
            v_last = dv[:, :, CH - 1].T.reshape(NT)
    C = (np.log(Vs[:-1]) - np.log(Us[1:])).sum()
    logZ = np.log((v_last * np.exp(etrans.astype(np.float64))).sum()) + C + c_off
    return logZ


def _score(emit, y, trans, strans, etrans):
    y = y.astype(np.int64)
    return (float(strans[y[0]])
            + trans[y[:-1], y[1:]].astype(np.float64).sum()
            + float(etrans[y[-1]])
            + emit[np.arange(T), y].astype(np.float64).sum())


def _ensure_axon_hooks():
    """Some images lack antenv.axon_hooks; bass_utils imports it whenever
    BASS_TRACE is set under axon.  Provide a no-op shim so kernel() never
    crashes on that path (tracing degrades gracefully)."""
    try:
        import antenv.axon_hooks  # noqa: F401
    except ImportError:
        import sys
        import types
        m = types.ModuleType("antenv.axon_hooks")
        state = {"v": None}
        m.get_axon_ntff_profile_hook = lambda: state["v"]
        m.set_axon_ntff_profile_hook = lambda v: state.update(v=v)
        sys.modules["antenv.axon_hooks"] = m


def kernel(emit, y, trans, strans, etrans):
    _ensure_axon_hooks()
    from concourse.bass_utils import run_bass_kernel_spmd

    emit = np.asarray(emit)
    trans = np.asarray(trans)
    strans = np.asarray(strans)
    etrans = np.asarray(etrans)
    y = np.asarray(y)

    nc = _get_nc()
    in_maps, c_off = _preprocess(emit, trans, strans)
    res = run_bass_kernel_spmd(nc, in_maps, list(range(NCORES)))
    _CACHE["last_res"] = res
    logZ = _postprocess(res.results, etrans, c_off)
    out = logZ - _score(emit, y, trans, strans, etrans)
    return np.asarray(out, dtype=np.float32)



# revision 10
# speedup vs baseline: 1.5036x; 1.5036x over previous
"""CRF forward (log-partition) kernel for Trainium2, 8 NeuronCores.

Problem: T=16384 steps, NT=1024 tags.
  alpha_0 = strans + emit[0]
  alpha_t[k] = emit[t,k] + logsumexp_j(alpha_{t-1}[j] + trans[j,k])
  out = logsumexp(alpha_{T-1} + etrans) - gold_path_score

Algorithm (validated in fp64/fp8 numpy prototype; see baseline docstring in
kernel_bf16_baseline.py for the chunked-scan derivation):
  Work in exp space: with Mc[k]=max_j trans[j,k], Ehat=exp(trans-Mc) in (0,1],
  ghat_t = exp(emit[t]+Mc-mu_t), mu_t = max_k(emit[t]+Mc) + RBAR, the scan is a
  LINEAR recurrence  b_t = ghat_t * (Ehat^T b_{t-1})  whose scalar offsets are
  tracked exactly on the host.  Positive-matrix products contract directions
  (~20-30x/step), so the sequence is cut into 2048 chunks of L=8 steps; each
  chunk's chain starts 1 step early (burn-in) from ghat of that step, after
  which its direction matches the true state's.  Per-chunk unknown scalars are
  recovered on the host by telescoping ratios of dumped seam vectors.  Chain 0
  is exact (b_0 injected via a K=1 bf16 matmul patch into PSUM).

  Speed: matmuls run in fp8 e4m3 with perf_mode=DoubleRow — each instruction
  contracts K=256 (two 128-blocks packed per PE cell, 2 MACs/cell/cycle), so a
  micro-step is 8 output blocks x 4 pair-matmuls of [128x(2x128)] weights
  against the [128x(2x256)] moving b tile, accumulating fp32 in PSUM, then 8
  DVE multiplies by bf16 ghat writing the next fp8 b tile.  256 chains/core x
  8 cores, W=9 micro-steps, no inter-core communication.  Measured sustained
  rate ~125-131 ns per DR matmul (microbench) vs ~79 ns for the bf16 N=128
  baseline's 2x-more matmuls.

Chain schedule: chain i>=1 covers steps [8i, 8i+8); micro-step tau processes
step 8i + tau - 2 (tau=1 is burn-in).  Chain 0 covers steps 1..7: ghat=0
padding until tau=2, when the K=1 patch matmul injects b_0 (chain 0's ghat
column is 1.0 at that step).  Each chain's start vector is ghat of its first
processed step (the tau=1 g tile quantized to fp8 = the initial b).  Dumps
(fp8 b-tiles): u at tau=1 (state entering own chunk), v at tau=W (state
entering next chunk).  Host fixup (fp64): C_0=0,
C_i = C_{i-1} + log sum(v_{i-1}) - log sum(u_i);
logZ = log(v_last . exp(etrans)) + C_last + c_off.
"""

import numpy as np

T, NT = 16384, 1024
NCORES = 8
CH = 256            # chains per core
L = 16384 // (NCORES * CH)   # chunk length = 8
W = L + 1           # micro-steps; burn-in = 1
PATCH_TAU = 2       # patch applied at this micro-step
U_TAU = 1
V_TAU = W
RBAR = 1.0          # per-step growth fold-in; centers the per-chain scale
                    # drift (measured [3.9e-3, 64] over a chunk) in e5m2 range
N_WARMUP = 48       # junk matmuls to trip the HAM clock-gate during init DMA

_CACHE = {}


def _build_nc():
    import concourse.bass as bass
    import concourse.mybir as mybir
    import concourse.tile as tile
    from concourse import bacc

    nc = bacc.Bacc("TRN2", target_bir_lowering=False, debug=False,
                   num_devices=NCORES)
    bf16 = mybir.dt.bfloat16
    f32 = mybir.dt.float32
    f8w = mybir.dt.float8e4      # weights: e4m3 precision
    f8b = mybir.dt.float8e5      # moving b: e5m2 range (chain scale drifts
                                 # ~10 logs over a chunk; e4m3 underflows)
    DR = mybir.MatmulPerfMode.DoubleRow

    EH = nc.dram_tensor("ehat", [8, 128, 8, 128], f8w, kind="ExternalInput")
    GH = nc.dram_tensor("ghat", [W, 128, 8 * CH], bf16, kind="ExternalInput")
    BI = nc.dram_tensor("binit", [128, 8 * CH], f8b, kind="ExternalInput")
    PA = nc.dram_tensor("patch", [1, 1024], bf16, kind="ExternalInput")
    DU = nc.dram_tensor("du", [128, 8 * CH], f8b, kind="ExternalOutput")
    DV = nc.dram_tensor("dv", [128, 8 * CH], f8b, kind="ExternalOutput")

    with tile.TileContext(nc) as tc:
        with (
            tc.tile_pool(name="const", bufs=1) as const,
            tc.tile_pool(name="bpool", bufs=2) as bpool,
            tc.tile_pool(name="gpool", bufs=4) as gpool,
            tc.tile_pool(name="psum", bufs=6, space="PSUM") as psum,
            tc.tile_pool(name="wpsum", bufs=1, space="PSUM") as wpsum,
        ):
            # PE warm-up: junk matmuls with no DMA dependency so the HAM
            # un-throttles while the init DMAs stream in.
            wsrc = const.tile([128, 128], bf16)
            nc.any.memset(wsrc[:], 0.25)
            wps = wpsum.tile([128, 128], f32)
            for _ in range(N_WARMUP):
                nc.tensor.matmul(wps[:], lhsT=wsrc[:], rhs=wsrc[:],
                                 start=True, stop=True)

            # Init loads in parallel (HWDGE shares bandwidth; serializing
            # costs DMA-start latency per group).
            binit_sb = const.tile([128, 8, CH], f8b)
            nc.sync.dma_start(binit_sb[:], BI[:])
            ehat_sb = []
            g_tiles = {}
            for mth in range(8):
                t = const.tile([128, 8, 128], f8w, tag=f"eh{mth}")
                nc.sync.dma_start(t[:], EH[mth])
                ehat_sb.append(t)
                if mth == 0:
                    for gi in (1, 2):
                        gt = gpool.tile([128, 8, CH], bf16, tag="g")
                        nc.sync.dma_start(gt[:], GH[gi - 1])
                        g_tiles[gi] = gt
            cur_b = binit_sb
            # Patch is injected as a K=1 outer-product bf16 matmul into PSUM
            # at PATCH_TAU (ps += patch_row^T @ onehot), with the host setting
            # chain 0's ghat column to 1.0 at that step.
            patch_sb = const.tile([1, 1024], bf16)
            nc.sync.dma_start(patch_sb[:], PA[:])
            onehot = const.tile([1, CH], bf16)
            nc.any.memset(onehot[:], 0.0)
            nc.any.memset(onehot[0:1, 0:1], 1.0)

            for tau in range(1, W + 1):
                g_tile = g_tiles.pop(tau)
                if tau + 2 <= W:
                    nt = gpool.tile([128, 8, CH], bf16, tag="g")
                    nc.sync.dma_start(nt[:], GH[tau + 1])
                    g_tiles[tau + 2] = nt
                new_b = bpool.tile([128, 8, CH], f8b, tag="b")
                for mth in range(8):
                    ps = psum.tile([128, CH], f32, tag="ps")
                    for q in range(4):
                        nc.tensor.matmul(
                            ps[:],
                            lhsT=ehat_sb[mth][:, 2 * q:2 * q + 2, :],
                            rhs=cur_b[:, 2 * q:2 * q + 2, :],
                            start=(q == 0),
                            stop=(q == 3 and tau != PATCH_TAU),
                            perf_mode=DR)
                    if tau == PATCH_TAU:
                        nc.tensor.matmul(
                            ps[:],
                            lhsT=patch_sb[:, mth * 128:(mth + 1) * 128],
                            rhs=onehot[:],
                            start=False, stop=True)
                    nc.vector.tensor_tensor(
                        out=new_b[:, mth, :], in0=ps[:],
                        in1=g_tile[:, mth, :],
                        op=mybir.AluOpType.mult)
                if tau == U_TAU:
                    nc.sync.dma_start(DU[:], new_b[:])
                if tau == V_TAU:
                    nc.sync.dma_start(DV[:], new_b[:])
                cur_b = new_b

    nc.compile()
    return nc


def _get_nc():
    if "nc" not in _CACHE:
        _CACHE["nc"] = _build_nc()
    return _CACHE["nc"]


def _chain_steps():
    """steps[i, tau-1] = global step processed by chain i at micro-step tau
    (-1 = pad)."""
    steps = np.full((NCORES * CH, W), -1, dtype=np.int64)
    taus = np.arange(1, W + 1)
    steps[0] = np.where(taus > PATCH_TAU, taus - PATCH_TAU, -1)
    idx = np.arange(1, NCORES * CH)
    steps[1:] = L * idx[:, None] + taus[None, :] - PATCH_TAU
    return steps


def _preprocess(emit, trans, strans):
    import ml_dtypes
    bf16 = ml_dtypes.bfloat16
    f8w = ml_dtypes.float8_e4m3
    f8b = ml_dtypes.float8_e5m2

    emit64 = emit.astype(np.float64)
    trans64 = trans.astype(np.float64)
    Mc = trans64.max(axis=0)
    Ehat = np.exp(trans64 - Mc[None, :]).astype(np.float32)
    # eh[mth, p, jc, q] = Ehat[jc*128+p, mth*128+q]
    eh = np.ascontiguousarray(
        Ehat.reshape(8, 128, 8, 128).transpose(2, 1, 0, 3)
    ).astype(f8w)

    A = emit64 + Mc[None, :]
    mu = A.max(axis=1) + RBAR                       # [T]
    ghat = np.exp(A - mu[:, None]).astype(np.float32)   # [T, NT]

    a0 = strans.astype(np.float64) + emit64[0]
    c0 = a0.max()
    b0 = np.exp(a0 - c0).astype(np.float32)
    c_off = c0 + mu[1:].sum()

    steps = _chain_steps()
    in_maps = []
    for c in range(NCORES):
        S = steps[c * CH:(c + 1) * CH]              # [CH, W]
        G = ghat[np.clip(S, 0, T - 1)]              # [CH, W, NT]
        G = np.where((S >= 1)[:, :, None], G, 0.0)
        if c == 0:
            # chain 0 pad: ghat=1 at PATCH_TAU so the PSUM-injected patch
            # passes through the multiply unchanged
            G[0, PATCH_TAU - 1, :] = 1.0
        # GH[tau, p, blk*CH+ch] = ghat[t_ch, blk*128+p]
        Gt = (G.transpose(1, 2, 0)                  # [W, NT, CH]
                .reshape(W, 8, 128, CH)
                .transpose(0, 2, 1, 3)
                .reshape(W, 128, 8 * CH))
        gh = np.ascontiguousarray(Gt).astype(bf16)
        bi = Gt[0].astype(np.float32).astype(f8b)

        pa = np.zeros((1, 1024), np.float32)
        if c == 0:
            pa[0] = b0                              # k-ordered patch row
        in_maps.append({"ehat": np.asarray(eh),
                        "ghat": np.asarray(gh),
                        "binit": np.asarray(bi),
                        "patch": pa.astype(bf16)})
    return in_maps, c_off


def _postprocess(results, etrans, c_off):
    """Telescoping seam corrections in fp64."""
    n = NCORES * CH
    Us = np.zeros(n)
    Vs = np.zeros(n)
    v_last = None
    for c in range(NCORES):
        du = results[c]["du"].astype(np.float64).reshape(128, 8, CH)
        dv = results[c]["dv"].astype(np.float64).reshape(128, 8, CH)
        Us[c * CH:(c + 1) * CH] = du.sum(axis=(0, 1))
        Vs[c * CH:(c + 1) * CH] = dv.sum(axis=(0, 1))
        if c == NCORES - 1:
            # v[k = blk*128+p] of last chain = dv[p, blk, CH-1]
            v_last = dv[:, :, CH - 1].T.reshape(NT)
    C = (np.log(Vs[:-1]) - np.log(Us[1:])).sum()
    logZ = np.log((v_last * np.exp(etrans.astype(np.float64))).sum()) + C + c_off
    return logZ


def _score(emit, y, trans, strans, etrans):
    y = y.astype(np.int64)
    return (float(strans[y[0]])
            + trans[y[:-1], y[1:]].astype(np.float64).sum()
            + float(etrans[y[-1]])
            + emit[np.arange(T), y].astype(np.float64).sum())


def _ensure_axon_hooks():
    """Some images lack antenv.axon_hooks; bass_utils imports it whenever
    BASS_TRACE is set under axon.  Provide a no-op shim so kernel() never
    crashes on that path (tracing degrades gracefully)."""
    try:
        import antenv.axon_hooks  # noqa: F401
    except ImportError:
        import sys
        import types
        m = types.ModuleType("antenv.axon_hooks")
        state = {"v": None}
        m.get_axon_ntff_profile_hook = lambda: state["v"]
        m.set_axon_ntff_profile_hook = lambda v: state.update(v=v)
        sys.modules["antenv.axon_hooks"] = m


def kernel(emit, y, trans, strans, etrans):
    _ensure_axon_hooks()
    from concourse.bass_utils import run_bass_kernel_spmd

    emit = np.asarray(emit)
    trans = np.asarray(trans)
    strans = np.asarray(strans)
    etrans = np.asarray(etrans)
    y = np.asarray(y)

    nc = _get_nc()
    in_maps, c_off = _preprocess(emit, trans, strans)
    res = run_bass_kernel_spmd(nc, in_maps, list(range(NCORES)))
    _CACHE["last_res"] = res
    logZ = _postprocess(res.results, etrans, c_off)
    out = logZ - _score(emit, y, trans, strans, etrans)
    return np.asarray(out, dtype=np.float32)


# revision 13
# speedup vs baseline: 1.6937x; 1.1265x over previous
"""CRF forward (log-partition) kernel for Trainium2, 8 NeuronCores.

Problem: T=16384 steps, NT=1024 tags.
  alpha_0 = strans + emit[0]
  alpha_t[k] = emit[t,k] + logsumexp_j(alpha_{t-1}[j] + trans[j,k])
  out = logsumexp(alpha_{T-1} + etrans) - gold_path_score

Algorithm (validated in fp64/fp8 numpy prototype):
  Work in exp space: with Mc[k]=max_j trans[j,k], Ehat=exp(trans-Mc) in (0,1],
  ghat_t = exp(emit[t]+Mc-mu_t), mu_t = max_k(emit[t]+Mc) + RBAR, the scan is a
  LINEAR recurrence  b_t = ghat_t * (Ehat^T b_{t-1})  whose scalar offsets are
  tracked exactly on the host.  The sequence is cut into 2048 chunks of L=8
  steps; chunk i's chain starts from the seed ghat[8i] standing in for
  b_{8i-1} (positive-matrix products contract directions ~20-30x/step, so by
  the chunk's end the direction is true).  Per-chunk unknown scalars are
  recovered on the host by telescoping ratios: the seed u_i is host-known
  exactly (it IS the fp8 b_init input), v_i is the dumped end-of-chunk state.
  Chain 0 is exact (b_0 injected via a K=1 bf16 matmul patch into PSUM at
  tau=1; its seed column is zero so only the patch contributes).

  Speed: matmuls run fp8 with perf_mode=DoubleRow — each instruction
  contracts K=256 (two 128-blocks packed per PE cell, 2 MACs/cell/cycle):
  per micro-step, 8 output blocks x 4 pair-matmuls of [128x(2x128)] e4m3
  weights against the [128x(2x256)] e5m2 moving b tile (e5m2 because the
  per-chain scale drifts ~10 logs over a chunk, overflowing e4m3's range),
  accumulating fp32 in PSUM, then 8 DVE multiplies by e4m3 ghat writing the
  next e5m2 b tile.  256 chains/core x 8 cores, W=8 micro-steps, no
  inter-core communication.  b tiles are split into two half-tiles (j-blocks
  0-3 / 4-7) so the next micro-step's first matmuls only wait on half the
  DVE writes.  Measured sustained rate ~124 ns per DR matmul.

Chain schedule: chain i>=1 covers steps [8i, 8i+8) at taus 1..8 (no burn-in
micro-step; the seed itself is u_i).  Chain 0 covers steps 1..7 at taus
2..8, patch at tau=1.  Host fixup (fp64): C_0=0,
C_i = C_{i-1} + log sum(v_{i-1}) - log sum(u_i);
logZ = log(v_last . exp(etrans)) + C_last + c_off.
"""

import numpy as np

T, NT = 16384, 1024
NCORES = 8
CH = 256            # chains per core
L = 16384 // (NCORES * CH)   # chunk length = 8
W = L               # micro-steps (no burn-in)
PATCH_TAU = 1       # patch applied at this micro-step
V_TAU = W
RBAR = 1.0          # per-step growth fold-in; centers the per-chain scale
                    # drift (measured [3.9e-3, 64] over a chunk) in e5m2 range
N_WARMUP = 24       # junk matmuls to trip the HAM clock-gate during init DMA

_CACHE = {}


def _build_nc():
    import concourse.bass as bass
    import concourse.mybir as mybir
    import concourse.tile as tile
    from concourse import bacc

    nc = bacc.Bacc("TRN2", target_bir_lowering=False, debug=False,
                   num_devices=NCORES)
    bf16 = mybir.dt.bfloat16
    f32 = mybir.dt.float32
    f8w = mybir.dt.float8e4      # weights + ghat: e4m3 precision
    f8b = mybir.dt.float8e5      # moving b: e5m2 range
    DR = mybir.MatmulPerfMode.DoubleRow

    EH = nc.dram_tensor("ehat", [8, 128, 8, 128], f8w, kind="ExternalInput")
    GH = nc.dram_tensor("ghat", [W, 128, 8 * CH], f8w, kind="ExternalInput")
    BI = nc.dram_tensor("binit", [2, 128, 4 * CH], f8b, kind="ExternalInput")
    PA = nc.dram_tensor("patch", [1, 1024], bf16, kind="ExternalInput")
    DV = nc.dram_tensor("dv", [2, 128, 4 * CH], f8b, kind="ExternalOutput")

    with tile.TileContext(nc) as tc:
        with (
            tc.tile_pool(name="const", bufs=1) as const,
            tc.tile_pool(name="bpool", bufs=2) as bpool,
            tc.tile_pool(name="gpool", bufs=4) as gpool,
            tc.tile_pool(name="psum", bufs=6, space="PSUM") as psum,
            tc.tile_pool(name="wpsum", bufs=1, space="PSUM") as wpsum,
        ):
            # PE warm-up: junk matmuls with no DMA dependency so the HAM
            # un-throttles while the init DMAs stream in.
            wsrc = const.tile([128, 128], bf16)
            nc.any.memset(wsrc[:], 0.25)
            wps = wpsum.tile([128, 128], f32)
            for _ in range(N_WARMUP):
                nc.tensor.matmul(wps[:], lhsT=wsrc[:], rhs=wsrc[:],
                                 start=True, stop=True)

            # Init loads, ordered so the first micro-step's operands land
            # first: b-init half A, eh0, b-init half B, g1, eh1-7, g2.
            binit_sb = [const.tile([128, 4, CH], f8b, tag=f"bi{h}",
                                   name=f"binit{h}") for h in range(2)]
            ehat_sb = [const.tile([128, 8, 128], f8w, tag=f"eh{m}",
                                  name=f"ehat{m}") for m in range(8)]
            nc.sync.dma_start(binit_sb[0][:], BI[0])
            nc.sync.dma_start(ehat_sb[0][:], EH[0])
            nc.sync.dma_start(binit_sb[1][:], BI[1])
            g_tiles = {1: gpool.tile([128, 8, CH], f8w, tag="g",
                                     name="g1")}
            nc.sync.dma_start(g_tiles[1][:], GH[0])
            for mth in range(1, 8):
                nc.sync.dma_start(ehat_sb[mth][:], EH[mth])
            g_tiles[2] = gpool.tile([128, 8, CH], f8w, tag="g",
                                    name="g2")
            nc.sync.dma_start(g_tiles[2][:], GH[1])
            patch_sb = const.tile([1, 1024], bf16)
            nc.sync.dma_start(patch_sb[:], PA[:])
            onehot = const.tile([1, CH], bf16)
            nc.any.memset(onehot[:], 0.0)
            nc.any.memset(onehot[0:1, 0:1], 1.0)

            cur_b = binit_sb
            for tau in range(1, W + 1):
                g_tile = g_tiles.pop(tau)
                if tau + 2 <= W:
                    nt = gpool.tile([128, 8, CH], f8w, tag="g",
                                    name=f"g{tau + 2}")
                    nc.sync.dma_start(nt[:], GH[tau + 1])
                    g_tiles[tau + 2] = nt
                new_b = [bpool.tile([128, 4, CH], f8b, tag=f"b{h}",
                                    name=f"b{tau}_{h}") for h in range(2)]
                for mth in range(8):
                    ps = psum.tile([128, CH], f32, tag="ps")
                    for q in range(4):
                        nc.tensor.matmul(
                            ps[:],
                            lhsT=ehat_sb[mth][:, 2 * q:2 * q + 2, :],
                            rhs=cur_b[q // 2][:, 2 * (q % 2):2 * (q % 2) + 2, :],
                            start=(q == 0),
                            stop=(q == 3 and tau != PATCH_TAU),
                            perf_mode=DR)
                    if tau == PATCH_TAU:
                        nc.tensor.matmul(
                            ps[:],
                            lhsT=patch_sb[:, mth * 128:(mth + 1) * 128],
                            rhs=onehot[:],
                            start=False, stop=True)
                    nc.vector.tensor_tensor(
                        out=new_b[mth // 4][:, mth % 4, :], in0=ps[:],
                        in1=g_tile[:, mth, :],
                        op=mybir.AluOpType.mult)
                if tau == V_TAU:
                    nc.sync.dma_start(DV[0], new_b[0][:])
                    nc.sync.dma_start(DV[1], new_b[1][:])
                cur_b = new_b

    nc.compile()
    return nc


def _get_nc():
    if "nc" not in _CACHE:
        _CACHE["nc"] = _build_nc()
    return _CACHE["nc"]


def _chain_steps():
    """steps[i, tau-1] = global step processed by chain i at micro-step tau
    (-1 = pad)."""
    steps = np.full((NCORES * CH, W), -1, dtype=np.int64)
    taus = np.arange(1, W + 1)
    steps[0] = np.where(taus > PATCH_TAU, taus - PATCH_TAU, -1)
    idx = np.arange(1, NCORES * CH)
    steps[1:] = L * idx[:, None] + taus[None, :] - PATCH_TAU
    return steps


def _preprocess(emit, trans, strans):
    import ml_dtypes
    bf16 = ml_dtypes.bfloat16
    f8w = ml_dtypes.float8_e4m3
    f8b = ml_dtypes.float8_e5m2

    emit64 = emit.astype(np.float64)
    trans64 = trans.astype(np.float64)
    Mc = trans64.max(axis=0)
    Ehat = np.exp(trans64 - Mc[None, :]).astype(np.float32)
    # eh[mth, p, jc, q] = Ehat[jc*128+p, mth*128+q]
    eh = np.ascontiguousarray(
        Ehat.reshape(8, 128, 8, 128).transpose(2, 1, 0, 3)
    ).astype(f8w)

    A = emit64 + Mc[None, :]
    mu = A.max(axis=1) + RBAR                       # [T]
    ghat = np.exp(A - mu[:, None]).astype(np.float32)   # [T, NT]

    a0 = strans.astype(np.float64) + emit64[0]
    c0 = a0.max()
    b0 = np.exp(a0 - c0).astype(np.float32)
    c_off = c0 + mu[1:].sum()

    steps = _chain_steps()
    in_maps = []
    us_all = np.zeros(NCORES * CH)
    for c in range(NCORES):
        S = steps[c * CH:(c + 1) * CH]              # [CH, W]
        G = ghat[np.clip(S, 0, T - 1)]              # [CH, W, NT]
        G = np.where((S >= 1)[:, :, None], G, 0.0)
        if c == 0:
            # chain 0 pad: ghat=1 at PATCH_TAU so the PSUM-injected patch
            # passes through the multiply unchanged
            G[0, PATCH_TAU - 1, :] = 1.0
        # GH[tau, p, blk*CH+ch] = ghat[t_ch, blk*128+p]
        Gt = (G.transpose(1, 2, 0)                  # [W, NT, CH]
                .reshape(W, 8, 128, CH)
                .transpose(0, 2, 1, 3)
                .reshape(W, 128, 8 * CH))
        gh = np.ascontiguousarray(Gt).astype(f8w)
        # seeds: e5m2-quantized tau-1 ghat columns; chain 0's seed is zero
        # (the patch matmul ADDS b_0 into PSUM, so any seed would leak).
        bi32 = Gt[0].astype(np.float32)
        if c == 0:
            bi32.reshape(128, 8, CH)[:, :, 0] = 0.0
        bi = bi32.astype(f8b)
        # u_i = seed sums, computed exactly from the fp8 input itself
        us_all[c * CH:(c + 1) * CH] = (
            bi.astype(np.float64).reshape(128, 8, CH).sum(axis=(0, 1)))

        pa = np.zeros((1, 1024), np.float32)
        if c == 0:
            pa[0] = b0                              # k-ordered patch row
        bi_split = np.ascontiguousarray(
            np.asarray(bi).reshape(128, 2, 4 * CH).transpose(1, 0, 2))
        in_maps.append({"ehat": np.asarray(eh),
                        "ghat": np.asarray(gh),
                        "binit": bi_split,
                        "patch": pa.astype(bf16)})
    return in_maps, c_off, us_all


def _postprocess(results, etrans, c_off, us_all):
    """Telescoping seam corrections in fp64."""
    n = NCORES * CH
    Vs = np.zeros(n)
    v_last = None
    for c in range(NCORES):
        dv = (results[c]["dv"].astype(np.float64)
              .reshape(2, 128, 4, CH).transpose(1, 0, 2, 3)
              .reshape(128, 8, CH))
        Vs[c * CH:(c + 1) * CH] = dv.sum(axis=(0, 1))
        if c == NCORES - 1:
            # v[k = blk*128+p] of last chain = dv[p, blk, CH-1]
            v_last = dv[:, :, CH - 1].T.reshape(NT)
    C = (np.log(Vs[:-1]) - np.log(us_all[1:])).sum()
    logZ = np.log((v_last * np.exp(etrans.astype(np.float64))).sum()) + C + c_off
    return logZ


def _score(emit, y, trans, strans, etrans):
    y = y.astype(np.int64)
    return (float(strans[y[0]])
            + trans[y[:-1], y[1:]].astype(np.float64).sum()
            + float(etrans[y[-1]])
            + emit[np.arange(T), y].astype(np.float64).sum())


def _ensure_axon_hooks():
    """Some images lack antenv.axon_hooks; bass_utils imports it whenever
    BASS_TRACE is set under axon.  Provide a no-op shim so kernel() never
    crashes on that path (tracing degrades gracefully)."""
    try:
        import antenv.axon_hooks  # noqa: F401
    except ImportError:
        import sys
        import types
        m = types.ModuleType("antenv.axon_hooks")
        state = {"v": None}
        m.get_axon_ntff_profile_hook = lambda: state["v"]
        m.set_axon_ntff_profile_hook = lambda v: state.update(v=v)
        sys.modules["antenv.axon_hooks"] = m


def kernel(emit, y, trans, strans, etrans):
    _ensure_axon_hooks()
    from concourse.bass_utils import run_bass_kernel_spmd

    emit = np.asarray(emit)
    trans = np.asarray(trans)
    strans = np.asarray(strans)
    etrans = np.asarray(etrans)
    y = np.asarray(y)

    nc = _get_nc()
    in_maps, c_off, us_all = _preprocess(emit, trans, strans)
    res = run_bass_kernel_spmd(nc, in_maps, list(range(NCORES)))
    _CACHE["last_res"] = res
    logZ = _postprocess(res.results, etrans, c_off, us_all)
    out = logZ - _score(emit, y, trans, strans, etrans)
    return np.asarray(out, dtype=np.float32)
